# revision 54
# baseline (speedup 1.0000x reference)
"""Trainium2 Bass kernel v2: faithful-reshape causal attention.

Host-side prep (inside kernel(), numpy): per-block x^T in bf16, Wqkv/Wo
pre-cast to bf16 in [128, 8, N] row-chunk layout. Device per (b, h) block
(x rows [128h, 128h+128) of batch b):
  qkv = x @ Wqkv: 3 gemm-pairs (2x512-col psum groups per [128,1024] tile).
  set1: 16 transposes -> kq [128, 2048]: rows 0:64 q^T, 64:128 k^T,
        col = 128r + a (packed 2x evicts).
  set2: 8 paired-r strided transposes -> vT [64, 2048] col = 128r + a.
  vaug: 16 transposes -> va [128 kpos, 16, 65] bf16, ones at col 64.
  S^T strips (kpos block j vs qpos >= 128j) in [128,1024] psum tiles,
  one exp per tile on ACT -> ptb bf16; diagonal affine_select on Pool.
  PV^T: out [128 qpos, 65] per strip j accumulating kpos i2 <= j
        (65-col matmuls; col 64 = softmax denominator).
  norm: batched reciprocal + per-strip tensor_scalar fused
        normalize+evict -> nrm bf16 [128 qpos, 64].
  re-transpose 4 strips/psum tile -> Tall [64 e, 2048 qpos].
  wl [(rpar, e), k, a] via 2 strided copies; y = wl^T @ Wo halves -> DMA.
32 independent blocks; 8 cores x 4 blocks, zero collectives. All DMA via
SP-issued HWDGE.
"""
import sys

sys.path.insert(0, '/opt/trn_rl_repo')

import numpy as np

B, L, D = 2, 2048, 1024
H = 16
RB = 128
D3 = 3 * D
NB = 4
NCORES = 8
P = 128
NKB = 16

STRIP_W = [2048 - 128 * i for i in range(NKB)]
STRIP_OFF = [0] * NKB
for _i in range(1, NKB):
    STRIP_OFF[_i] = STRIP_OFF[_i - 1] + STRIP_W[_i - 1]
PT_TOTAL = STRIP_OFF[-1] + STRIP_W[-1]   # 17408

_cached = {}


def _build_program():
    import concourse.bass as bass
    import concourse.mybir as mybir
    import concourse.tile as tile
    from concourse.tile import add_dep_helper

    f32 = mybir.dt.float32
    bf16 = mybir.dt.bfloat16
    EXP = mybir.ActivationFunctionType.Exp
    GE = mybir.AluOpType.is_ge

    nc = bass.Bass()
    xt = nc.declare_dram_parameter("xt", [P, NB, 8, P], bf16, isOutput=False)
    wq = nc.declare_dram_parameter("wq", [P, 8, D3], bf16, isOutput=False)
    wo = nc.declare_dram_parameter("wo", [P, 8, D], bf16, isOutput=False)
    ys = nc.declare_dram_parameter("ys", [NB, RB, D], f32, isOutput=True)

    with tile.TileContext(nc) as tc:
        with (
            tc.tile_pool(name="const", bufs=1) as constp,
            tc.tile_pool(name="wq", bufs=1) as wqp,
            tc.tile_pool(name="wop", bufs=1) as wop,
            tc.tile_pool(name="xtp", bufs=1) as xtp,
            tc.tile_pool(name="qkvp", bufs=2) as qkvp,
            tc.tile_pool(name="qkt", bufs=2) as qktp,
            tc.tile_pool(name="vtt", bufs=1) as vtp,
            tc.tile_pool(name="vap", bufs=2) as vap,
            tc.tile_pool(name="ptp", bufs=2) as ptp,
            tc.tile_pool(name="nrm", bufs=8) as nrmp,
            tc.tile_pool(name="rcp", bufs=2) as rcpp,
            tc.tile_pool(name="tal", bufs=2) as talp,
            tc.tile_pool(name="wl", bufs=4) as wlp,
            tc.tile_pool(name="yo", bufs=2) as yop,
            tc.tile_pool(name="ps", bufs=2, space="PSUM") as psp,     # S/gemm
            tc.tile_pool(name="pp", bufs=2, space="PSUM") as ppp,     # PV grp
            tc.tile_pool(name="tr", bufs=2, space="PSUM") as trp,     # tr / Y
        ):
            def absorb_on(eng, *prods):
                # Walrus caps every instruction at ONE sync wait. Emit
                # queue-local nops that sync-depend on each producer; the
                # post-pass elides waits covered by these earlier nops.
                for p in prods:
                    if p is None:
                        continue
                    n = eng.nop(hint="dep")
                    add_dep_helper(n.ins, p.ins, sync=True)

            def absorb(*prods):
                absorb_on(nc.tensor, *prods)

            # rotating-pool WAR bookkeeping: readers[n] collects handles of
            # ops reading tile n; a new tile absorbs the readers of the tile
            # it displaces.
            def make_pool(alloc, depth):
                readers = []
                cnt = [0]

                def get():
                    n = cnt[0]
                    if n >= depth:
                        absorb(*readers[n - depth])
                    cnt[0] += 1
                    readers.append([])
                    return alloc(), n
                return get, readers

            ps_get, ps_readers = make_pool(
                lambda: psp.tile([P, 1024], f32, tag="ps", name="pstile"), 2)
            pp_get, pp_readers = make_pool(
                lambda: ppp.tile([P, 512], f32, tag="pp", name="pptile"), 2)
            tr_get, tr_readers = make_pool(
                lambda: trp.tile([P, 1024], bf16, tag="tr", name="trtile"), 2)

            dma_hs = []

            def dma(dst, src):
                h = nc.sync.dma_start(dst, src)
                dma_hs.append(h)
                return h

            identb_t = constp.tile([P, 128], bf16, tag="identb")
            identb = identb_t[:, 0:128]
            h_idb = nc.gpsimd.memset(identb, 0.0)
            absorb_on(nc.gpsimd, h_idb)
            h_idb2 = nc.gpsimd.affine_select(
                out=identb, in_=identb, compare_op=mybir.AluOpType.not_equal,
                fill=1.0, base=0, pattern=[[-1, 128]], channel_multiplier=1)

            # ---- input DMAs (SP HWDGE). W fully resident in bf16.
            xT = xtp.tile([P, NB, 8, P], bf16, tag="xT")
            wq_sb = wqp.tile([P, 8, D3], bf16, tag="wq")
            h_xt = {0: dma(xT[:, 0], xt[:, 0])}
            h_wch = {}
            h_c0 = [dma(wq_sb[:, 2 * q:2 * q + 2, 0:512],
                        wq[:, 2 * q:2 * q + 2, 0:512]) for q in range(4)]
            h_wch[0] = h_c0[3]
            h_xt[1] = dma(xT[:, 1], xt[:, 1])
            h_wch[1] = dma(wq_sb[:, :, 512:1024], wq[:, :, 512:1024])
            for c in range(2, 4):
                h_wch[c] = dma(wq_sb[:, :, 512 * c:512 * c + 512],
                               wq[:, :, 512 * c:512 * c + 512])
            h_wch[4] = dma(wq_sb[:, :, 2048:2560], wq[:, :, 2048:2560])
            h_xt[2] = dma(xT[:, 2], xt[:, 2])
            h_wch[5] = dma(wq_sb[:, :, 2560:3072], wq[:, :, 2560:3072])
            h_xt[3] = dma(xT[:, 3], xt[:, 3])
            wo_sb = wop.tile([P, 8, D], bf16, tag="wo")
            h_wo = {}

            def emit_wo_dma(h):
                h_wo[h] = dma(wo_sb[:, :, 512 * h:512 * h + 512],
                              wo[:, :, 512 * h:512 * h + 512])

            # ---- gemm pairs
            blk = {}          # per-block setup state
            qkv_of = {}       # i -> qkv tile
            ev_h = {}         # (c2, i) -> evict handle
            ev_flip = [0]

            pair_st = {}

            def gemm_half(c2, i, half):
                if i not in qkv_of:
                    qkv_of[i] = qkvp.tile([P, D3], bf16, tag="qkv",
                                          name="qkv")
                    # WAR: reuses block i-2's qkv, last read by its set2
                    # transposes (PE) - same engine, in-order, no sync.
                qkv = qkv_of[i]
                c = 2 * c2 + half
                if half == 0:
                    if c == 0 and i == 0:
                        absorb(h_c0[0], h_xt[i])
                    else:
                        absorb(h_wch[c], h_xt[i])
                    # noqa: gemm psum tile below
                    pair_st[(c2, i)] = ps_get()
                else:
                    absorb(h_wch[c])
                qp, n = pair_st[(c2, i)]
                h_mm = None
                for k in range(8):
                    if c == 0 and i == 0 and k in (2, 4, 6):
                        absorb(h_c0[k // 2])
                    h_mm = nc.tensor.matmul(
                        qp[:, 512 * half:512 * half + 512],
                        xT[:, i, k, :],
                        wq_sb[:, k, 512 * c:512 * c + 512],
                        start=(k == 0), stop=(k == 7))
                if half == 1:
                    eng = nc.vector
                    absorb_on(eng, h_mm)
                    st = blk.get(i - 2)
                    if st is not None:
                        absorb_on(eng, st.get("h_set2_t"))
                    h_ev = eng.tensor_copy(
                        qkv[:, 1024 * c2:1024 * c2 + 1024], qp[:, 0:1024])
                    ps_readers[n].append(h_ev)
                    ev_h[(2 * c2, i)] = h_ev
                    ev_h[(2 * c2 + 1, i)] = h_ev

            def gemm_pair(c2, i):
                gemm_half(c2, i, 0)
                gemm_half(c2, i, 1)

            # ---- per-block q/k/v transposition
            blk_pv_last = {}
            blk_s_last = {}
            blk_aff_last = {}

            def setup_piece(i, piece):
                st = blk.setdefault(i, {})
                qkv = qkv_of[i]
                qk_v = qkv[:, 0:2048].rearrange("p (r c) -> p r c", c=128)
                v_v = qkv[:, 2048:D3].rearrange("p (m c) -> p m c", c=128)
                if piece in (0, 1):
                    # set1 half g: q|k columns, 8 r-groups. q^T and k^T go
                    # to separate base-0 tiles (matmul needs lhsT/rhs at the
                    # same base partition).
                    g = piece
                    if g == 0:
                        absorb(ev_h[(0, i)], ev_h[(1, i)])
                        st["qT"] = qktp.tile([64, 2048], bf16, tag="qT",
                                             name="qT")
                        st["kT"] = qktp.tile([64, 2048], bf16, tag="kT",
                                             name="kT")
                    else:
                        absorb(ev_h[(2, i)], ev_h[(3, i)])
                    tp, n = tr_get()
                    h_t = None
                    for t in range(8):
                        r = 8 * g + t
                        h_t = nc.tensor.transpose(
                            tp[:, 128 * t:128 * t + 128],
                            qk_v[:, r, :], identb)
                    kTv = st["kT"].rearrange("p (a r) -> p r a", r=16)
                    absorb_on(nc.vector, h_t, blk_s_last.get(i - 2))
                    h_q = nc.vector.tensor_copy(
                        st["qT"][:, 1024 * g:1024 * g + 1024],
                        tp[0:64, 0:1024])
                    h_k = nc.vector.tensor_copy(
                        kTv[:, 8 * g:8 * g + 8, :],
                        tp[64:128].rearrange("p (t a) -> p t a", t=8))
                    tr_readers[n].extend([h_q, h_k])
                    st.setdefault("h_kq", []).extend([h_q, h_k])
                elif piece == 2:
                    # set2: v columns, one r-pair (128 contiguous cols) per
                    # transpose
                    absorb(ev_h[(4, i)], ev_h[(5, i)])
                    st["vT"] = vtp.tile([64, 2048], bf16, tag="vT", name="vT")
                    tp, n = tr_get()
                    h_t = None
                    for m in range(8):
                        h_t = nc.tensor.transpose(
                            tp[:, 128 * m:128 * m + 128],
                            v_v[:, m, :], identb)
                    st["h_set2_t"] = h_t
                    tps = tp.rearrange("p (m a) -> p m a", m=8)
                    vTv = st["vT"].rearrange("p (a r) -> p r a", r=16)
                    # vT bufs=1: block i's evicts overwrite block i-1's vT,
                    # whose readers were its vaug transposes (PE).
                    prev_vaug = blk.get(i - 1, {}).get("h_vaug_t")
                    absorb_on(nc.vector, h_t, prev_vaug)
                    h_v0 = nc.vector.tensor_copy(vTv[:, 0:16:2, :], tps[0:64])
                    h_v1 = nc.vector.tensor_copy(vTv[:, 1:16:2, :],
                                                 tps[64:128])
                    tr_readers[n].extend([h_v0, h_v1])
                    st["h_vT"] = [h_v0, h_v1]
                else:
                    # piece 3 - vaug: 16 kpos blocks -> va [128, 16, 65]
                    st["va"] = vap.tile([P, NKB, 65], bf16, tag="va",
                                        name="va")
                    absorb_on(nc.gpsimd, blk_pv_last.get(i - 2))
                    st["h_ms"] = nc.gpsimd.memset(st["va"][:, :, 64:65], 1.0)
                    vTa = st["vT"].rearrange("p (a r) -> p a r", r=16)
                    tp, n = tr_get()
                    absorb(st["h_vT"][0], st["h_vT"][1])
                    h_t = None
                    for i2 in range(NKB):
                        h_t = nc.tensor.transpose(
                            tp[:, 64 * i2:64 * i2 + 64],
                            vTa[0:64, 8 * i2:8 * i2 + 8, :],
                            identb[0:64, 0:64])
                    st["h_vaug_t"] = h_t
                    absorb_on(nc.vector, h_t, st["h_ms"],
                              blk_pv_last.get(i - 2))
                    h_e = nc.vector.tensor_copy(
                        st["va"][:, :, 0:64],
                        tp.rearrange("p (s e) -> p s e", s=16))
                    tr_readers[n].append(h_e)
                    st["h_va"] = h_e

            # ---- phase B
            bst = {}
            retr_hist = []
            wl_hist = []
            y_hist = []
            rcp_hist = []

            def start_block(i):
                bst[i] = {
                    "ptb": ptp.tile([P, PT_TOTAL], bf16, tag="ptb",
                                    name="ptb"),
                    "qTa": blk[i]["qT"].rearrange("p (r a) -> p a r", r=16),
                    "kTa": blk[i]["kT"].rearrange("p (a r) -> p a r", r=16),
                    "strip_done": [None] * NKB,
                    "pp": [None] * 4,
                    "ppn": [0] * 4,
                    "pv_h": [None] * NKB,
                    "nrm": [None] * NKB,
                    "h_nrm": [None] * NKB,
                    "tal": talp.tile([64, 2048], bf16, tag="tal", name="tal"),
                    "h_tal": [None] * 4,
                }

            def strip_tiles(i, j):
                """Yield after each <=1024-col S tile so other PE work can
                interleave between tiles (softens the 2-deep ps rotation)."""
                st_i = bst[i]
                ptb = st_i["ptb"]
                qTa = st_i["qTa"]
                lhsT = st_i["kTa"][:, 8 * j:8 * j + 8, :]
                w = STRIP_W[j]
                off = STRIP_OFF[j]
                col = 0
                first = True
                while col < w:
                    pw = min(1024, w - col)
                    stile, n = ps_get()
                    h_mm = None
                    for u in range(0, pw, 512):
                        uw = min(512, pw - u)
                        a0 = 8 * j + (col + u) // 16
                        h_mm = nc.tensor.matmul(
                            stile[:, u:u + uw], lhsT,
                            qTa[:, a0:a0 + uw // 16, :],
                            start=True, stop=True)
                    blk_s_last[i] = h_mm
                    if first:
                        absorb_on(nc.scalar, blk_pv_last.get(i - 2),
                                  blk_aff_last.get(i - 2))
                    if pw > 512:
                        absorb_on(nc.scalar, h_mm)
                    h_exp = nc.scalar.activation(
                        ptb[:, off + col:off + col + pw], stile[:, 0:pw],
                        EXP, scale=0.25)
                    ps_readers[n].append(h_exp)
                    if first:
                        absorb_on(nc.gpsimd, h_exp, blk_pv_last.get(i - 2))
                        h_aff = nc.gpsimd.affine_select(
                            out=ptb[:, off:off + 128],
                            in_=ptb[:, off:off + 128],
                            compare_op=GE, fill=0.0, base=0,
                            pattern=[[1, 128]], channel_multiplier=-1)
                        blk_aff_last[i] = h_aff
                        st_i["strip_aff"] = st_i.get("strip_aff", {})
                        st_i["strip_aff"][j] = h_aff
                        first = False
                    st_i["strip_done"][j] = h_exp
                    col += pw
                    yield

            def emit_pv(i, j):
                st_i = bst[i]
                st = blk[i]
                ptb = st_i["ptb"]
                g = j // 4
                if j % 4 == 0:
                    st_i["pp"][g], st_i["ppn"][g] = pp_get()
                pt = st_i["pp"][g]
                absorb(st_i["strip_done"][j], st_i["strip_aff"][j],
                       st["h_va"] if j == 0 else None)
                h_pv = None
                for i2 in range(j + 1):
                    h_pv = nc.tensor.matmul(
                        pt[:, 65 * (j % 4):65 * (j % 4) + 65],
                        ptb[:, STRIP_OFF[i2] + 128 * (j - i2):
                            STRIP_OFF[i2] + 128 * (j - i2) + 128],
                        st["va"][:, i2, :],
                        start=(i2 == 0), stop=(i2 == j))
                blk_pv_last[i] = h_pv
                st_i["pv_h"][j] = h_pv

            def emit_norm(i, g):
                st_i = bst[i]
                pt = st_i["pp"][g]
                ptr = pt[:, 0:260].rearrange("p (s c) -> p s c", c=65)
                rcp = rcpp.tile([P, 4], f32, tag="rcp")
                absorb_on(nc.vector, st_i["pv_h"][4 * g + 3])
                if len(rcp_hist) >= 2:
                    absorb_on(nc.vector, *rcp_hist[-2])
                h_rcp = nc.vector.reciprocal(rcp[:], ptr[:, 0:4, 64])
                rs = []
                for s in range(4):
                    j = 4 * g + s
                    eng = nc.vector
                    nb = nrmp.tile([P, 256], bf16, tag="nrm",
                                   name="nrm")[:, 0:64]
                    absorb_on(eng, h_rcp)
                    if len(retr_hist) >= 2:
                        absorb_on(eng, retr_hist[-2])
                    h_n = eng.tensor_scalar_mul(
                        nb, ptr[:, s, 0:64], rcp[:, s:s + 1])
                    pp_readers[st_i["ppn"][g]].append(h_n)
                    st_i["nrm"][j] = nb
                    st_i["h_nrm"][j] = h_n
                    rs.append(h_n)
                rcp_hist.append(rs)

            def emit_retr(i, g):
                st_i = bst[i]
                tp, n = tr_get()
                h_t = None
                for s in range(4):
                    j = 4 * g + s
                    absorb(st_i["h_nrm"][j])
                    h_t = nc.tensor.transpose(
                        tp[0:64, 128 * s:128 * s + 128],
                        st_i["nrm"][j], identb)
                retr_hist.append(h_t)
                absorb_on(nc.vector, h_t)
                if g == 0 and len(wl_hist) >= 2:
                    absorb_on(nc.vector, *wl_hist[-2])
                h_e = nc.vector.tensor_copy(
                    st_i["tal"][:, 512 * g:512 * g + 512], tp[0:64, 0:512])
                tr_readers[n].append(h_e)
                st_i["h_tal"][g] = h_e
                # stream this group's wl slices now (keeps the block tail off
                # the critical path)
                if g == 0:
                    st_i["wl"] = wlp.tile([P, 8, P], bf16, tag="wl",
                                          name="wl")
                tala = st_i["tal"].rearrange("p (a k rp) -> p k rp a",
                                             k=8, rp=2)
                ws = []
                for rp in range(2):
                    eng = nc.vector if rp == 0 else nc.gpsimd
                    absorb_on(eng, h_e)
                    prev = tail_h.get((i - 2, 1))
                    if g == 0 and prev is not None:
                        absorb_on(eng, prev[0])
                    h = eng.tensor_copy(
                        st_i["wl"][64 * rp:64 * rp + 64, :,
                                   32 * g:32 * g + 32],
                        tala[0:64, :, rp, 32 * g:32 * g + 32])
                    ws.append(h)
                st_i.setdefault("h_wlparts", []).extend(ws)
                if g == 3:
                    wl_hist.append(st_i["h_wlparts"])
                    st_i["h_wl"] = st_i["h_wlparts"][-2:]

            tail_h = {}

            def emit_tail_half(i, h2):
                st_i = bst[i]
                wl = st_i["wl"]
                yp, n = tr_get()
                ypf = yp[:, 0:1024].bitcast(f32)
                absorb(st_i["h_wl"][0], st_i["h_wl"][1], h_wo.get(h2))
                h_mm = None
                for k in range(8):
                    h_mm = nc.tensor.matmul(
                        ypf, wl[:, k, :],
                        wo_sb[:, k, 512 * h2:512 * h2 + 512],
                        start=(k == 0), stop=(k == 7))
                y_sb = yop.tile([P, 512], f32, tag="y", name="y_sb")
                eng = nc.vector
                absorb_on(eng, h_mm)
                if len(y_hist) >= 2:
                    absorb_on(eng, y_hist[-2])
                h_ye = eng.tensor_copy(y_sb[:], ypf)
                tr_readers[n].append(h_ye)
                absorb_on(nc.sync, h_ye)
                h_yd = dma(ys[i][:, 512 * h2:512 * h2 + 512], y_sb[:])
                y_hist.append(h_yd)
                tail_h[(i, h2)] = (h_mm, h_ye, h_yd)

            # ================= schedule =================
            # Startup: with q|k columns in chunks 0-3, strips start right
            # after pair 1 + set1; pair 2 (v) and set2/vaug interleave into
            # the first strips.
            gemm_pair(0, 0)
            setup_piece(0, 0)
            gemm_pair(0, 1)
            gemm_pair(1, 0)
            setup_piece(0, 1)
            gemm_pair(1, 1)
            emit_wo_dma(0)
            emit_wo_dma(1)
            start_block(0)

            # fillers keep the PE fed between S strips so ACT/DVE backlogs
            # never stall it: one ~0.4-1.7us PE unit per strip step.
            fillers = {}
            # block 0 own tail work: pair 2 (v cols) + set2 + vaug early
            fillers[(0, 0)] = lambda: gemm_half(2, 0, 0)
            fillers[(0, 1)] = lambda: gemm_half(2, 0, 1)
            fillers[(0, 2)] = lambda: setup_piece(0, 2)
            fillers[(0, 3)] = lambda: setup_piece(0, 3)
            # gemm halves for the next blocks: blocks 1/2/3 spread so every
            # strip window keeps ~1.7us of non-ACT PE work per step
            HP = [(c2, hf) for c2 in range(3) for hf in range(2)]
            gh_slots = (
                [((0, 8), 1, *HP[4]), ((0, 9), 1, *HP[5])] +
                [((0, 12), 2, *HP[0]), ((0, 13), 2, *HP[1])] +
                [((1, s), 2, *HP[2 + k]) for k, s in enumerate((0, 1, 2, 3))] +
                [((2, s), 3, *HP[k]) for k, s in
                 enumerate((0, 1, 2, 3, 4, 5))])
            for key, tgt, c2, hf in gh_slots:
                fillers[key] = \
                    (lambda c2=c2, tgt=tgt, hf=hf: gemm_half(c2, tgt, hf))
            # Wo tails pushed late (block-3 strips are otherwise ACT-bound)
            for k, (ti, s) in enumerate((
                    (0, (3, 0)), (0, (3, 1)), (1, (3, 4)), (1, (3, 5)),
                    (2, (3, 8)), (2, (3, 9)))):
                fillers[s] = (lambda ti=ti, k=k: emit_tail_half(ti, k % 2))
            # setup pieces for the next block
            setup_slots = {1: ((0, 10), (0, 11), (0, 14)),
                           2: ((1, 8), (1, 9), (1, 14)),
                           3: ((2, 8), (2, 9), (2, 14))}
            for tgt, (s0, s1, s2) in setup_slots.items():
                fillers[s0] = (lambda tgt=tgt: setup_piece(tgt, 0))
                fillers[s1] = (lambda tgt=tgt: setup_piece(tgt, 1))
                fillers[s2] = (lambda tgt=tgt: setup_piece(tgt, 2))

            pv_q = []
            retr_q = []
            step_n = [0]

            def pump_pv():
                pi, pj = pv_q.pop(0)
                emit_pv(pi, pj)
                if pj % 4 == 3:
                    emit_norm(pi, pj // 4)
                    retr_q.append((pi, pj // 4, step_n[0]))

            def pump_retr(force=False):
                if not retr_q:
                    return
                ri, rg, st0 = retr_q[0]
                if not force and step_n[0] - st0 < 3:
                    return
                retr_q.pop(0)
                emit_retr(ri, rg)

            for i in range(NB):
                for j in range(NKB):
                    step_n[0] += 1
                    work = [lambda: None]
                    if retr_q:
                        work.append(pump_retr)
                    f = fillers.get((i, j))
                    if f is not None:
                        work.append(f)
                    gen = strip_tiles(i, j)
                    for _ in gen:
                        if work:
                            work.pop(0)()
                        pv_here = (len(pv_q) > 5)
                        if pv_here:
                            pump_pv()
                    for wk in work:
                        wk()
                    pv_q.append((i, j))
                    if len(pv_q) > 6:
                        pump_pv()
                    if i == NB - 1 and j >= 12 and pv_q:
                        pump_pv()
                        if retr_q:
                            pump_retr()
                    if j == 15 and i + 1 < NB:
                        setup_piece(i + 1, 3)
                        start_block(i + 1)
            done_wl = [False]

            def try_tail():
                if len(bst[NB - 1].get("h_wlparts", ())) == 8 \
                        and not done_wl[0]:
                    done_wl[0] = True
                    emit_tail_half(NB - 1, 0)
                    emit_tail_half(NB - 1, 1)
            while pv_q:
                pump_pv()
                if retr_q:
                    pump_retr(force=True)
                try_tail()
            while retr_q:
                pump_retr(force=True)
                try_tail()
            try_tail()

            absorb_on(nc.sync, *dma_hs)
            absorb_on(nc.sync, h_idb2, blk_aff_last.get(NB - 1),
                      bst[NB - 1]["strip_done"][NKB - 1],
                      tail_h[(NB - 1, 1)][0], tail_h[(NB - 1, 1)][1],
                      blk_pv_last.get(NB - 1))

    return nc


def _elide_covered_waits(nc):
    """Walrus rejects >1 sync wait per instruction. Each queue's sequencer
    processes waits in dispatch order, so a wait already issued earlier in
    the same queue gates every later instruction in that queue. Drop only
    waits covered by an earlier same-queue wait (incl. absorber nops);
    own-queue waits are kept (dropping them races on real hardware) unless
    an instruction still exceeds one wait."""
    observed = {}
    leftover = []
    for inst in nc.all_instructions():
        si = inst.sync_info
        if si is None:
            continue
        if type(inst).__name__ in ("InstEventSemaphore", "InstTrigger"):
            continue
        eng = str(inst.engine)
        obs = observed.setdefault(eng, {})
        ow = list(si.on_wait or [])
        keep = [w for w in ow if obs.get(w.id, -1) < w.wait_value]
        for w in keep:
            obs[w.id] = max(obs.get(w.id, -1), w.wait_value)
        if len(keep) > 1:
            own = eng.split(".")[-1] + "_"
            keep2 = [w for w in keep if not w.ant_name.startswith(own)]
            if len(keep2) < len(keep):
                keep = keep2
        if len(keep) != len(ow):
            si.on_wait = keep
            inst.sync_info = si
        if len(keep) > 1:
            leftover.append((inst.name, type(inst).__name__, eng,
                             [(w.ant_name, w.wait_value) for w in keep]))
    if leftover:
        import logging
        logging.warning("multi-wait instructions remain: %s", leftover[:12])


def _get_program():
    if "nc" not in _cached:
        nc = _build_program()
        _elide_covered_waits(nc)
        _cached["nc"] = nc
    return _cached["nc"]


def kernel(x=None, mask=None, Wqkv=None, Wo=None, **_ignored):
    """Full inputs -> full output. mask ignored (guaranteed causal tril)."""
    from concourse.bass_utils import run_bass_kernel_spmd
    import ml_dtypes

    bf = ml_dtypes.bfloat16
    x = np.asarray(x, dtype=np.float32)
    Wqkv = np.asarray(Wqkv, dtype=np.float32)
    Wo = np.asarray(Wo, dtype=np.float32)

    # Column permutation: all q|k columns first (16 r-groups of 128),
    # then v columns (16 r-groups of 64). wq[p, k, n] = Wqkv'[128k + p, n].
    perm = np.concatenate(
        [np.arange(192 * r, 192 * r + 128) for r in range(16)] +
        [np.arange(192 * r + 128, 192 * r + 192) for r in range(16)])
    Wp = Wqkv[:, perm]
    wq_h = np.ascontiguousarray(
        Wp.reshape(8, P, D3).transpose(1, 0, 2).astype(bf))
    wo_h = np.ascontiguousarray(
        Wo.reshape(8, P, D).transpose(1, 0, 2).astype(bf))

    nc = _get_program()
    in_maps = []
    for c in range(NCORES):
        xts = np.empty((P, NB, 8, P), dtype=bf)
        for idx, g in enumerate(range(NB * c, NB * c + NB)):
            b, h = divmod(g, H)
            xb = x[b, RB * h:RB * h + RB, :]          # [128 a, 1024 d]
            # xt[p, i, k, a] = xb[a, 128k + p]
            xts[:, idx] = xb.T.reshape(8, P, P).transpose(1, 0, 2).astype(bf)
        in_maps.append({"xt": xts, "wq": wq_h, "wo": wo_h})

    res = run_bass_kernel_spmd(nc, in_maps, core_ids=list(range(NCORES)))
    y = np.empty((B, L, D), dtype=np.float32)
    for c in range(NCORES):
        ysc = res.results[c]["ys"]
        for idx, g in enumerate(range(NB * c, NB * c + NB)):
            b, h = divmod(g, H)
            y[b, RB * h:RB * h + RB, :] = ysc[idx]
    return y
